# revision 37
# baseline (speedup 1.0000x reference)
"""ArcMargin head (ArcFace) distributed over 8 TRN2 NeuronCores.

Strategy (classification / tensor parallel):
  - weight [C, D] is sharded along C across the 8 cores (12500 classes each,
    zero-padded to 12544 = 98*128 for uniform tiling); embeddings and the
    per-row gathered weight rows (weight[labels]) are replicated.
  - Key algebraic fact: the ArcFace margin (phi) only replaces the ONE target
    element per row; every other element of the [B, C] output is just
    SCALE * cosine.  So each device computes out = (SCALE*e_hat) @ w_hat.T
    with a bf16 TensorEngine matmul (f32 accumulate) and separately computes
    the target values 64*phi into a tiny [128, 16] tensor; the host places
    those 2048 values into the gathered output (pure indexing — the values
    themselves are computed on device).
  - Normalization (x / max(||x||, eps)), the *64 logit scale and the f32->bf16
    cast are fused into the per-tile load pipeline; the weight shard is
    transposed on-chip with TensorEngine transpose ops so the big matmul can
    stream [D, C]-major bf16 tiles.
  - DMA plumbing (the real bottleneck beyond the PE roofline):
      * output is written in bf16 (halves write traffic; host upcasts),
      * weight loads go on the sync HWDGE ring, embedding loads + all output
        stores on the scalar HWDGE ring, so load-completion semaphores never
        serialize behind store descriptors,
      * embeddings are loaded with batch rows permuted (row = p*16 + g) so
        each partition reads 8KB contiguous runs (4x fewer DMA descriptors);
        the store access pattern un-permutes rows for free,
      * stores are grouped 4 row-chunks per DMA into a single [B, CSP] bf16
        output tensor,
      * ramped column-block widths (256..2048) so dense matmul work starts
        after ~2MB of weight DMA instead of ~8MB.
"""

import math
import sys

import numpy as np

for _p in ("/opt/trn_rl_repo",):
    if _p not in sys.path:
        sys.path.append(_p)

import concourse.bass as bass
import concourse.tile as tile
from concourse import bacc
from concourse import mybir
from concourse.bass_utils import run_bass_kernel_spmd

SCALE = 64.0
MARGIN = 0.5
COS_M = math.cos(MARGIN)
SIN_M = math.sin(MARGIN)
TH = math.cos(math.pi - MARGIN)
MM = math.sin(math.pi - MARGIN) * MARGIN

B, D, C = 2048, 512, 100000
N_CORES = 8
CS = C // N_CORES          # 12500 real classes per core
CSP = 12544                # padded classes per core (98 * 128)
OOB = 1 << 30              # gather offset sentinel for "not my row"

F32 = mybir.dt.float32
BF16 = mybir.dt.bfloat16
I32 = mybir.dt.int32
AF = mybir.ActivationFunctionType
ALU = mybir.AluOpType


def build_program(b=B, d=D, csp=CSP):
    """Build the (SPMD-uniform) single-core Bass program."""
    mb = b // 128          # batch row-chunks
    kc = d // 128          # contraction chunks
    nc = bacc.Bacc()

    emb_d = nc.declare_dram_parameter("emb", [b, d], F32, isOutput=False)
    wsh_d = nc.declare_dram_parameter("wsh", [csp, d], F32, isOutput=False)
    goff_d = nc.declare_dram_parameter("goff", [128, mb], I32, isOutput=False)
    ident_d = nc.declare_dram_parameter("ident", [128, 128], F32, isOutput=False)
    out_d = nc.declare_dram_parameter("out", [b, csp], BF16, isOutput=True)
    tval_d = nc.declare_dram_parameter("tval", [128, mb], F32, isOutput=True)

    # batch row r lives at (partition p, chunk g) with r = p*16 + g, so each
    # partition's embedding DMA reads contiguous 8KB runs
    emb_pg = emb_d[:].rearrange("(p g) d -> p g d", g=mb)
    # store view: row-chunk g of partition p un-permutes to DRAM row p*16+g
    out_pg = out_d[:].rearrange("(p g) c -> p g c", g=mb)

    # ramped column blocks: small blocks first so dense matmul work starts
    # after only ~2MB of weight DMA (the head is DMA-bound: the 4.2MB of e +
    # the first blocks must stream in before the PE can go dense), and a
    # small final block so the evac+store tail is short
    # wide-ish first blocks: per-m matmul work must exceed the e-chunk
    # arrival cadence or the PE idles between chunks during the head
    widths = [1024, 1024, 2048, 2048, 2048, 2048, 1024, 1024, 256]
    assert sum(widths) == csp
    ctbs = []
    c0 = 0
    for w in widths:
        ctbs.append((c0, w))
        c0 += w
    max_cbw = max(w for _, w in ctbs)

    EGRP = 4          # emb row-chunks per DMA (1MB, 8KB runs)
    WGRP = 4          # weight row-chunks per DMA (1MB)
    SGRP = 4          # row-chunks per output store DMA

    with tile.TileContext(nc) as tc:
        with (
            tc.tile_pool(name="const", bufs=1) as constp,
            tc.tile_pool(name="persist", bufs=1) as persist,
            tc.tile_pool(name="ld", bufs=3) as ldp,
            tc.tile_pool(name="bf", bufs=3) as bfp,
            tc.tile_pool(name="scr", bufs=2) as scrp,
            tc.tile_pool(name="small", bufs=4) as smp,
            tc.tile_pool(name="wtb", bufs=2) as wtbp,
            tc.tile_pool(name="outp", bufs=3) as outp,
            tc.tile_pool(name="tpsum", bufs=2, space="PSUM") as tpsum,
            tc.tile_pool(name="cpsum", bufs=3, space="PSUM") as cpsum,
        ):
            # regular DMA + on-chip cast (the SWDGE cast path stalls ~15us on
            # gpsimd queue startup)
            ident_f = constp.tile([128, 128], F32, tag="ident_f")
            nc.sync.dma_start(out=ident_f[:], in_=ident_d[:])
            ident = constp.tile([128, 128], BF16)
            nc.vector.tensor_copy(out=ident[:], in_=ident_f[:])
            zb = constp.tile([128, 1], F32, tag="zb")
            nc.vector.memset(zb[:], 0.0)
            epsb = constp.tile([128, 1], F32, tag="epsb")
            nc.vector.memset(epsb[:], 1e-24)
            s2b = constp.tile([128, 1], F32, tag="s2b")
            nc.vector.memset(s2b[:], SCALE * SCALE)
            # preload the activation table set while the first DMAs stream
            # (the implicit ACT_TABLE_LOAD otherwise lands on the first
            # rownorm square, adding ~1.3us to the head critical path)
            warm = constp.tile([128, 1], F32, tag="warm")
            nc.scalar.activation(out=warm[:], in_=zb[:], func=AF.Square, bias=zb[:])
            nc.scalar.activation(out=warm[:], in_=zb[:], func=AF.Sqrt, bias=epsb[:])

            eT = persist.tile([128, kc, b], BF16)      # (64*e_hat)^T
            ebf_all = persist.tile([128, mb, d], BF16)  # 64*e_hat, p*16+g rows
            svec = persist.tile([128, mb], F32)        # 64*cos(target)
            gofft = persist.tile([128, mb], I32)       # gather row offsets
            tval = persist.tile([128, mb], F32)        # 64*phi / else-branch

            def rownorm_recip(x_f32, tag):
                # [128,1] f32 = 1 / max(||x_row||, ~1e-12)
                sq = scrp.tile([128, d], BF16, tag="sq_scr")
                ssq = smp.tile([128, 1], F32, tag=f"{tag}_ssq")
                nc.scalar.activation(
                    out=sq[:], in_=x_f32[:], func=AF.Square, bias=zb[:],
                    accum_out=ssq[:],
                )
                nrm = smp.tile([128, 1], F32, tag=f"{tag}_nrm")
                nc.scalar.activation(out=nrm[:], in_=ssq[:], func=AF.Sqrt, bias=epsb[:])
                rec = smp.tile([128, 1], F32, tag=f"{tag}_rec")
                nc.vector.reciprocal(out=rec[:], in_=nrm[:])
                return rec

            nc.sync.dma_start(out=gofft[:], in_=goff_d[:])
            evac_flip = [0]

            def emb_unit(m0, n, ui):
                # chunks m0..m0+n-1 (batch rows p*16+g); alternate HWDGE
                # rings so the e stream uses both descriptor generators
                eg = ldp.tile([128, EGRP, d], F32, tag="e_ld", name=f"eg_{m0}",
                              bufs=2)
                eng = nc.scalar if ui % 2 == 0 else nc.sync
                eng.dma_start(out=eg[:, :n, :], in_=emb_pg[:, m0:m0 + n, :])
                for g in range(n):
                    m = m0 + g
                    et = eg[:, g, :]
                    rec = rownorm_recip(et, "e")
                    rec64 = smp.tile([128, 1], F32, tag="e_rec64")
                    nc.scalar.mul(out=rec64[:], in_=rec[:], mul=SCALE)
                    ebf = ebf_all[:, m, :]
                    nc.vector.tensor_scalar_mul(out=ebf, in0=et, scalar1=rec64[:])
                    for k in range(kc):
                        pt = tpsum.tile([128, 128], BF16)
                        nc.tensor.transpose(
                            out=pt[:], in_=ebf[:, k * 128:(k + 1) * 128],
                            identity=ident[:],
                        )
                        nc.vector.tensor_copy(
                            out=eT[:, k, m * 128:(m + 1) * 128], in_=pt[:]
                        )

            def b_alloc(cb_idx):
                wtb = wtbp.tile(
                    [128, kc, max_cbw], BF16, tag="wtb", name=f"wtb_{cb_idx}"
                )
                wnb_all = wtbp.tile(
                    [128, max_cbw // 128, d], BF16, tag="wnb_all",
                    name=f"wnba_{cb_idx}", bufs=2,
                )
                return wtb, wnb_all

            def b_load_norm(cb_idx, wnb_all, cc, state):
                # one 128-row weight chunk: (load every WGRP), norm -> wnb_all
                cb0, cbw = ctbs[cb_idx]
                ncc = cbw // 128
                if cc % WGRP == 0:
                    ng = min(WGRP, ncc - cc)
                    state["wg"] = ldp.tile(
                        [128, WGRP, d], F32, tag="w_ld", name=f"wg_{cb_idx}_{cc}",
                        bufs=3,
                    )
                    nc.sync.dma_start(
                        out=state["wg"][:, :ng, :],
                        in_=wsh_d[
                            cb0 + cc * 128: cb0 + (cc + ng) * 128, :
                        ].rearrange("(g p) d -> p g d", p=128),
                    )
                wld = state["wg"][:, cc % WGRP, :]
                wrec = rownorm_recip(wld, "w")
                nc.vector.tensor_scalar_mul(
                    out=wnb_all[:, cc, :], in0=wld, scalar1=wrec[:]
                )

            def b_transpose(wnb_all, wtb, cc):
                for k in range(kc):
                    pt = tpsum.tile([128, 128], BF16)
                    nc.tensor.transpose(
                        out=pt[:], in_=wnb_all[:, cc, k * 128:(k + 1) * 128],
                        identity=ident[:],
                    )
                    nc.vector.tensor_copy(
                        out=wtb[:, k, cc * 128:(cc + 1) * 128], in_=pt[:]
                    )

            # wsel runs as a 2-stage pipeline: the indirect gather (gpsimd
            # SWDGE, ~2-4us latency) is issued >=2 chunks ahead of the
            # norm+dot so the in-order DVE/Act queues never head-of-line
            # block on gather data
            wsel_tiles = {}

            def wsel_gather(m):
                wt = ldp.tile([128, d], F32, tag="ws_ld", name=f"ws_{m}", bufs=4)
                nc.gpsimd.indirect_dma_start(
                    out=wt[:],
                    out_offset=None,
                    in_=wsh_d[:],
                    in_offset=bass.IndirectOffsetOnAxis(
                        ap=gofft[:, m:m + 1], axis=0
                    ),
                    bounds_check=csp - 1,
                    oob_is_err=False,
                )
                wsel_tiles[m] = wt

            def wsel_dot(m):
                wt = wsel_tiles.pop(m)
                wrec = rownorm_recip(wt, "ws")
                wnb = bfp.tile([128, d], BF16, tag="ws_bf")
                nc.vector.tensor_scalar_mul(out=wnb[:], in0=wt[:], scalar1=wrec[:])
                ttr_scr = scrp.tile([128, d], BF16, tag="ttr_scr")
                nc.vector.tensor_tensor(
                    out=ttr_scr[:], in0=ebf_all[:, m, :], in1=wnb[:], op=ALU.mult
                )
                nc.vector.tensor_reduce(
                    out=svec[:, m:m + 1], in_=ttr_scr[:],
                    axis=mybir.AxisListType.X, op=ALU.add,
                )

            # prologue: first e unit + B(0) only.  Remaining e units are
            # pulled lazily inside block 0's m-loop so block-0 matmuls
            # interleave with the e stream (PE executes in program order).
            wtb0, wnba0 = b_alloc(0)
            multi = len(ctbs) > 1
            wtb1, wnba1 = b_alloc(1) if multi else (None, None)
            st0 = {}
            ncc0 = ctbs[0][1] // 128
            ncc1 = ctbs[1][1] // 128 if multi else 0
            # a small first unit so the first chunk's norm+transpose chain
            # starts ~3us earlier; the rest in 1MB units
            e_units = [(0, 2), (2, 4), (6, 4), (10, 4), (14, 2)]
            emb_state = {"eidx": 0, "ui": 0}

            def pull_emb(upto_m):
                while emb_state["eidx"] <= upto_m and emb_state["ui"] < len(e_units):
                    m0, n = e_units[emb_state["ui"]]
                    emb_unit(m0, n, emb_state["ui"])
                    emb_state["ui"] += 1
                    emb_state["eidx"] = m0 + n

            pull_emb(0)
            for cc in range(ncc0):
                b_load_norm(0, wnba0, cc, st0)
            for cc in range(ncc0):
                b_transpose(wnba0, wtb0, cc)

            # ---------- main loop: C(cb) with B(cb+1) interleaved ----------
            wsel_done = 0
            wsel_issued = 0

            def phi_block():
                s2 = smp.tile([128, mb], F32, tag="s2")
                nc.scalar.activation(
                    out=s2[:], in_=svec[:], func=AF.Square, bias=zb[:]
                )
                rl = smp.tile([128, mb], F32, tag="rl")
                nc.scalar.activation(
                    out=rl[:], in_=s2[:], func=AF.Relu, bias=s2b[:], scale=-1.0
                )
                sn = smp.tile([128, mb], F32, tag="sn")
                nc.scalar.activation(out=sn[:], in_=rl[:], func=AF.Sqrt, bias=zb[:])
                pc = smp.tile([128, mb], F32, tag="pc")
                nc.vector.tensor_scalar_mul(out=pc[:], in0=svec[:], scalar1=COS_M)
                smt = smp.tile([128, mb], F32, tag="smt")
                nc.vector.tensor_scalar_mul(out=smt[:], in0=sn[:], scalar1=SIN_M)
                ph = smp.tile([128, mb], F32, tag="ph")
                nc.vector.tensor_tensor(
                    out=ph[:], in0=pc[:], in1=smt[:], op=ALU.subtract
                )
                eb = smp.tile([128, mb], F32, tag="eb")
                nc.vector.tensor_scalar_add(
                    out=eb[:], in0=svec[:], scalar1=-SCALE * MM
                )
                mk = smp.tile([128, mb], mybir.dt.uint8, tag="mk")
                nc.vector.tensor_scalar(
                    out=mk[:], in0=svec[:], scalar1=SCALE * TH, scalar2=None,
                    op0=ALU.is_gt,
                )
                nc.vector.select(out=tval[:], mask=mk[:], on_true=ph[:], on_false=eb[:])
                nc.scalar.dma_start(out=tval_d[:], in_=tval[:])

            cur_wtb = wtb0
            cur_wnba = wnba0
            for cb_idx, (cb0, cbw) in enumerate(ctbs):
                last_cb = cb_idx == len(ctbs) - 1
                nxt_wtb = nxt_wnba = None
                nxt_state = {}
                nxt_ncc = 0
                if cb_idx == 0:
                    nxt_wtb, nxt_wnba = wtb1, wnba1
                    nxt_ncc = ncc1
                elif not last_cb:
                    nxt_wtb, nxt_wnba = b_alloc(cb_idx + 1)
                    nxt_ncc = ctbs[cb_idx + 1][1] // 128

                nps = (cbw + 1023) // 1024   # 1024-wide psum tiles (2 banks)
                ot = None
                for m in range(mb):
                    if cb_idx == 0:
                        # stay ~one DMA unit ahead of the matmuls on the e
                        # stream (ring credits throttle runaway prefetch)
                        pull_emb(m + 5)
                    if m % SGRP == 0:
                        ot = outp.tile(
                            [128, SGRP, max_cbw], BF16, tag="o_t",
                            name=f"ot_{cb_idx}_{m}",
                        )
                    pss = [
                        cpsum.tile([128, 1024], F32, tag="mmps", name=f"mmps_{j}")
                        for j in range(nps)
                    ]
                    for k in range(kc):
                        for j5 in range((cbw + 511) // 512):
                            s0 = j5 * 512
                            sw = min(512, cbw - s0)
                            nc.tensor.matmul(
                                out=pss[j5 // 2][:, (j5 % 2) * 512:(j5 % 2) * 512 + sw],
                                lhsT=eT[:, k, m * 128:(m + 1) * 128],
                                rhs=cur_wtb[:, k, s0:s0 + sw],
                                start=(k == 0),
                                stop=(k == kc - 1),
                            )
                    for j in range(nps):
                        s0 = j * 1024
                        sw = min(1024, cbw - s0)
                        if evac_flip[0] % 2 == 0:
                            nc.vector.tensor_copy(
                                out=ot[:, m % SGRP, s0:s0 + sw], in_=pss[j][:, :sw]
                            )
                        else:
                            nc.scalar.copy(
                                out=ot[:, m % SGRP, s0:s0 + sw], in_=pss[j][:, :sw]
                            )
                        evac_flip[0] += 1
                    if m % SGRP == SGRP - 1:
                        # grouped store on the scalar HWDGE ring; the access
                        # pattern un-permutes rows (p,g) -> p*16+g
                        nc.scalar.dma_start(
                            out=out_pg[:, m - SGRP + 1:m + 1, cb0:cb0 + cbw],
                            in_=ot[:, :, :cbw],
                        )
                    # next block: loads+norms early (m 0..3).  PE transposes
                    # are emitted arrival-aware: the PE executes in order, so
                    # a transpose emitted before its chunk's DMA lands stalls
                    # the whole engine (and re-throttles it).  Start them only
                    # once the next block's bytes have had time to stream in.
                    if not last_cb:
                        if m < 4:
                            for cc in range(m * WGRP, min((m + 1) * WGRP, nxt_ncc)):
                                b_load_norm(cb_idx + 1, nxt_wnba, cc, nxt_state)
                        dma_ns = nxt_ncc * 873 + 5000
                        iter_ns = cbw * 1.71
                        m_start = max(4, int(dma_ns / iter_ns) + 1)
                        m_start = min(m_start, mb - (nxt_ncc + 1) // 2)
                        if m >= m_start:
                            # finish ~3 m-iterations before the block ends so
                            # the next block's first LDWEIGHTS never waits on
                            # the final transpose-copy chain
                            per_m = -(-nxt_ncc // max(1, mb - m_start - 3))
                            c0_ = (m - m_start) * per_m
                            for cc in range(c0_, min(c0_ + per_m, nxt_ncc)):
                                b_transpose(nxt_wnba, nxt_wtb, cc)
                    # sprinkle wsel chunks into the wide mid blocks (the ramp
                    # blocks' Act/DVE budget is fully claimed by norms+evacs);
                    # gathers run 2 chunks ahead of the norm+dot stage
                    if cb_idx >= 3 and m % 2 == 1:
                        if wsel_issued < mb:
                            wsel_gather(wsel_issued)
                            wsel_issued += 1
                        if wsel_done < wsel_issued - 2:
                            wsel_dot(wsel_done)
                            wsel_done += 1
                    if last_cb and m == 0:
                        while wsel_issued < mb:
                            wsel_gather(wsel_issued)
                            wsel_issued += 1
                        while wsel_done < mb:
                            wsel_dot(wsel_done)
                            wsel_done += 1
                        phi_block()
                cur_wtb = nxt_wtb
                cur_wnba = nxt_wnba

    nc.compile()
    return nc


_CACHE = {}


def _get_program():
    if "nc" not in _CACHE:
        _CACHE["nc"] = build_program()
    return _CACHE["nc"]


def make_in_maps(embeddings, labels, weight):
    embeddings = np.ascontiguousarray(np.asarray(embeddings, dtype=np.float32))
    weight = np.asarray(weight, dtype=np.float32)
    labels_np = np.asarray(labels).astype(np.int64)
    ident = np.eye(128, dtype=np.float32)
    # batch row r maps to (p, g) = (r // 16, r % 16)
    lab_pg = labels_np.reshape(128, B // 128)
    in_maps = []
    for k in range(N_CORES):
        wsh = np.zeros((CSP, D), np.float32)
        wsh[:CS] = weight[k * CS:(k + 1) * CS]
        own = (lab_pg // CS) == k
        col = lab_pg - k * CS
        goff_arr = np.ascontiguousarray(
            np.where(own, col, OOB).astype(np.int32)
        )
        in_maps.append(
            {"emb": embeddings, "wsh": wsh, "goff": goff_arr, "ident": ident}
        )
    return in_maps


def _gather(results, labels):
    labels_np = np.asarray(labels).astype(np.int64)
    full = np.empty((B, C), np.float32)
    for k in range(N_CORES):
        shard = np.asarray(results[k]["out"]).reshape(B, CSP)
        full[:, k * CS:(k + 1) * CS] = shard[:, :CS].astype(np.float32)
    # place the device-computed target values (64*phi) — indexing only
    rows = np.arange(B)
    cores = labels_np // CS
    for k in range(N_CORES):
        own = cores == k
        r = rows[own]
        tv = np.asarray(results[k]["tval"])  # [128, 16], row r at [r//16, r%16]
        full[r, labels_np[own]] = tv[r // 16, r % 16]
    return full


def kernel(embeddings, labels, weight):
    nc = _get_program()
    in_maps = make_in_maps(embeddings, labels, weight)
    res = run_bass_kernel_spmd(nc, in_maps, core_ids=list(range(N_CORES)))
    return _gather(res.results, labels)


def kernel_profiled(embeddings, labels, weight, **kw):
    """Like kernel() but also returns the BassKernelResults (exec_time_ns)."""
    nc = _get_program()
    in_maps = make_in_maps(embeddings, labels, weight)
    res = run_bass_kernel_spmd(
        nc, in_maps, core_ids=list(range(N_CORES)), trace=True, **kw
    )
    return _gather(res.results, labels), res


# revision 41
# speedup vs baseline: 1.2106x; 1.2106x over previous
"""ArcMargin head (ArcFace) distributed over 8 TRN2 NeuronCores.

Strategy (classification / tensor parallel):
  - weight [C, D] is sharded along C across the 8 cores (12500 classes each,
    zero-padded to 12544 = 98*128 for uniform tiling); embeddings and the
    per-row gathered weight rows (weight[labels]) are replicated.
  - Key algebraic fact: the ArcFace margin (phi) only replaces the ONE target
    element per row; every other element of the [B, C] output is just
    SCALE * cosine.  So each device computes out = (SCALE*e_hat) @ w_hat.T
    with a bf16 TensorEngine matmul (f32 accumulate) and separately computes
    the target values 64*phi into a tiny [128, 16] tensor; the host places
    those 2048 values into the gathered output (pure indexing — the values
    themselves are computed on device).
  - Normalization (x / max(||x||, eps)), the *64 logit scale and the f32->bf16
    cast are fused into the per-tile load pipeline; the weight shard is
    transposed on-chip with TensorEngine transpose ops so the big matmul can
    stream [D, C]-major bf16 tiles.
  - DMA plumbing (the real bottleneck beyond the PE roofline):
      * output is written in bf16 (halves write traffic; host upcasts),
      * weight loads go on the sync HWDGE ring, embedding loads + all output
        stores on the scalar HWDGE ring, so load-completion semaphores never
        serialize behind store descriptors,
      * embeddings are loaded with batch rows permuted (row = p*16 + g) so
        each partition reads 8KB contiguous runs (4x fewer DMA descriptors);
        the store access pattern un-permutes rows for free,
      * stores are grouped 4 row-chunks per DMA into a single [B, CSP] bf16
        output tensor,
      * ramped column-block widths (256..2048) so dense matmul work starts
        after ~2MB of weight DMA instead of ~8MB.
"""

import math
import sys

import numpy as np

for _p in ("/opt/trn_rl_repo",):
    if _p not in sys.path:
        sys.path.append(_p)

import concourse.bass as bass
import concourse.tile as tile
from concourse import bacc
from concourse import mybir
from concourse.bass_utils import run_bass_kernel_spmd

SCALE = 64.0
MARGIN = 0.5
COS_M = math.cos(MARGIN)
SIN_M = math.sin(MARGIN)
TH = math.cos(math.pi - MARGIN)
MM = math.sin(math.pi - MARGIN) * MARGIN

B, D, C = 2048, 512, 100000
N_CORES = 8
CS = C // N_CORES          # 12500 real classes per core
CSP = 12544                # padded classes per core (98 * 128)
OOB = 1 << 30              # gather offset sentinel for "not my row"

F32 = mybir.dt.float32
BF16 = mybir.dt.bfloat16
I32 = mybir.dt.int32
AF = mybir.ActivationFunctionType
ALU = mybir.AluOpType


def build_program(b=B, d=D, csp=CSP):
    """Build the (SPMD-uniform) single-core Bass program."""
    mb = b // 128          # batch row-chunks
    kc = d // 128          # contraction chunks
    nc = bacc.Bacc()

    emb_d = nc.declare_dram_parameter("emb", [b, d], F32, isOutput=False)
    wsh_d = nc.declare_dram_parameter("wsh", [csp, d], F32, isOutput=False)
    goff_d = nc.declare_dram_parameter("goff", [128, mb], I32, isOutput=False)
    ident_d = nc.declare_dram_parameter("ident", [128, 128], F32, isOutput=False)
    out_d = nc.declare_dram_parameter("out", [b, csp], BF16, isOutput=True)
    tval_d = nc.declare_dram_parameter("tval", [128, mb], F32, isOutput=True)

    # batch row r lives at (partition p, chunk g) with r = p*16 + g, so each
    # partition's embedding DMA reads contiguous 8KB runs
    emb_pg = emb_d[:].rearrange("(p g) d -> p g d", g=mb)
    # store view: row-chunk g of partition p un-permutes to DRAM row p*16+g
    out_pg = out_d[:].rearrange("(p g) c -> p g c", g=mb)

    # ramped column blocks: small blocks first so dense matmul work starts
    # after only ~2MB of weight DMA (the head is DMA-bound: the 4.2MB of e +
    # the first blocks must stream in before the PE can go dense), and a
    # small final block so the evac+store tail is short
    # wide-ish first blocks: per-m matmul work must exceed the e-chunk
    # arrival cadence or the PE idles between chunks during the head
    widths = [1024, 1024, 2048, 2048, 2048, 2048, 1024, 1024, 256]
    assert sum(widths) == csp
    ctbs = []
    c0 = 0
    for w in widths:
        ctbs.append((c0, w))
        c0 += w
    max_cbw = max(w for _, w in ctbs)

    EGRP = 4          # emb row-chunks per DMA (1MB, 8KB runs)
    WGRP = 4          # weight row-chunks per DMA (1MB)
    SGRP = 4          # row-chunks per output store DMA

    with tile.TileContext(nc) as tc:
        with (
            tc.tile_pool(name="const", bufs=1) as constp,
            tc.tile_pool(name="persist", bufs=1) as persist,
            tc.tile_pool(name="ld", bufs=3) as ldp,
            tc.tile_pool(name="bf", bufs=3) as bfp,
            tc.tile_pool(name="scr", bufs=2) as scrp,
            tc.tile_pool(name="small", bufs=4) as smp,
            tc.tile_pool(name="wtb", bufs=2) as wtbp,
            tc.tile_pool(name="outp", bufs=3) as outp,
            tc.tile_pool(name="tpsum", bufs=2, space="PSUM") as tpsum,
            tc.tile_pool(name="cpsum", bufs=6, space="PSUM") as cpsum,
        ):
            # regular DMA + on-chip cast (the SWDGE cast path stalls ~15us on
            # gpsimd queue startup)
            ident_f = constp.tile([128, 128], F32, tag="ident_f")
            nc.sync.dma_start(out=ident_f[:], in_=ident_d[:])
            ident = constp.tile([128, 128], BF16)
            nc.vector.tensor_copy(out=ident[:], in_=ident_f[:])
            zb = constp.tile([128, 1], F32, tag="zb")
            nc.vector.memset(zb[:], 0.0)
            epsb = constp.tile([128, 1], F32, tag="epsb")
            nc.vector.memset(epsb[:], 1e-24)
            s2b = constp.tile([128, 1], F32, tag="s2b")
            nc.vector.memset(s2b[:], SCALE * SCALE)
            # preload the activation table set while the first DMAs stream
            # (the implicit ACT_TABLE_LOAD otherwise lands on the first
            # rownorm square, adding ~1.3us to the head critical path)
            warm = constp.tile([128, 1], F32, tag="warm")
            nc.scalar.activation(out=warm[:], in_=zb[:], func=AF.Square, bias=zb[:])
            nc.scalar.activation(out=warm[:], in_=zb[:], func=AF.Sqrt, bias=epsb[:])

            eT = persist.tile([128, kc, b], BF16)      # (64*e_hat)^T
            ebf_all = persist.tile([128, mb, d], BF16)  # 64*e_hat, p*16+g rows
            svec = persist.tile([128, mb], F32)        # 64*cos(target)
            gofft = persist.tile([128, mb], I32)       # gather row offsets
            tval = persist.tile([128, mb], F32)        # 64*phi / else-branch

            def rownorm_recip(x_f32, tag):
                # [128,1] f32 = 1 / max(||x_row||, ~1e-12)
                sq = scrp.tile([128, d], BF16, tag="sq_scr")
                ssq = smp.tile([128, 1], F32, tag=f"{tag}_ssq")
                nc.scalar.activation(
                    out=sq[:], in_=x_f32[:], func=AF.Square, bias=zb[:],
                    accum_out=ssq[:],
                )
                nrm = smp.tile([128, 1], F32, tag=f"{tag}_nrm")
                nc.scalar.activation(out=nrm[:], in_=ssq[:], func=AF.Sqrt, bias=epsb[:])
                rec = smp.tile([128, 1], F32, tag=f"{tag}_rec")
                nc.vector.reciprocal(out=rec[:], in_=nrm[:])
                return rec

            nc.sync.dma_start(out=gofft[:], in_=goff_d[:])
            evac_flip = [0]

            def emb_unit(m0, n, ui):
                # chunks m0..m0+n-1 (batch rows p*16+g); alternate HWDGE
                # rings so the e stream uses both descriptor generators
                eg = ldp.tile([128, EGRP, d], F32, tag="e_ld", name=f"eg_{m0}",
                              bufs=2)
                eng = nc.scalar if ui % 2 == 0 else nc.sync
                eng.dma_start(out=eg[:, :n, :], in_=emb_pg[:, m0:m0 + n, :])
                for g in range(n):
                    m = m0 + g
                    et = eg[:, g, :]
                    rec = rownorm_recip(et, "e")
                    rec64 = smp.tile([128, 1], F32, tag="e_rec64")
                    nc.scalar.mul(out=rec64[:], in_=rec[:], mul=SCALE)
                    ebf = ebf_all[:, m, :]
                    nc.vector.tensor_scalar_mul(out=ebf, in0=et, scalar1=rec64[:])
                    for k in range(kc):
                        pt = tpsum.tile([128, 128], BF16)
                        nc.tensor.transpose(
                            out=pt[:], in_=ebf[:, k * 128:(k + 1) * 128],
                            identity=ident[:],
                        )
                        nc.vector.tensor_copy(
                            out=eT[:, k, m * 128:(m + 1) * 128], in_=pt[:]
                        )

            def b_alloc(cb_idx):
                wtb = wtbp.tile(
                    [128, kc, max_cbw], BF16, tag="wtb", name=f"wtb_{cb_idx}"
                )
                wnb_all = wtbp.tile(
                    [128, max_cbw // 128, d], BF16, tag="wnb_all",
                    name=f"wnba_{cb_idx}", bufs=2,
                )
                return wtb, wnb_all

            def b_load_norm(cb_idx, wnb_all, cc, state):
                # one 128-row weight chunk: (load every WGRP), norm -> wnb_all
                cb0, cbw = ctbs[cb_idx]
                ncc = cbw // 128
                if cc % WGRP == 0:
                    ng = min(WGRP, ncc - cc)
                    state["wg"] = ldp.tile(
                        [128, WGRP, d], F32, tag="w_ld", name=f"wg_{cb_idx}_{cc}",
                        bufs=3,
                    )
                    nc.sync.dma_start(
                        out=state["wg"][:, :ng, :],
                        in_=wsh_d[
                            cb0 + cc * 128: cb0 + (cc + ng) * 128, :
                        ].rearrange("(g p) d -> p g d", p=128),
                    )
                wld = state["wg"][:, cc % WGRP, :]
                wrec = rownorm_recip(wld, "w")
                nc.vector.tensor_scalar_mul(
                    out=wnb_all[:, cc, :], in0=wld, scalar1=wrec[:]
                )

            def b_transpose(wnb_all, wtb, cc):
                for k in range(kc):
                    pt = tpsum.tile([128, 128], BF16)
                    nc.tensor.transpose(
                        out=pt[:], in_=wnb_all[:, cc, k * 128:(k + 1) * 128],
                        identity=ident[:],
                    )
                    nc.vector.tensor_copy(
                        out=wtb[:, k, cc * 128:(cc + 1) * 128], in_=pt[:]
                    )

            # wsel runs as a 2-stage pipeline: the indirect gather (gpsimd
            # SWDGE, ~2-4us latency) is issued >=2 chunks ahead of the
            # norm+dot so the in-order DVE/Act queues never head-of-line
            # block on gather data
            wsel_tiles = {}

            def wsel_gather(m):
                wt = ldp.tile([128, d], F32, tag="ws_ld", name=f"ws_{m}", bufs=4)
                nc.gpsimd.indirect_dma_start(
                    out=wt[:],
                    out_offset=None,
                    in_=wsh_d[:],
                    in_offset=bass.IndirectOffsetOnAxis(
                        ap=gofft[:, m:m + 1], axis=0
                    ),
                    bounds_check=csp - 1,
                    oob_is_err=False,
                )
                wsel_tiles[m] = wt

            def wsel_dot(m):
                wt = wsel_tiles.pop(m)
                wrec = rownorm_recip(wt, "ws")
                wnb = bfp.tile([128, d], BF16, tag="ws_bf")
                nc.vector.tensor_scalar_mul(out=wnb[:], in0=wt[:], scalar1=wrec[:])
                ttr_scr = scrp.tile([128, d], BF16, tag="ttr_scr")
                nc.vector.tensor_tensor(
                    out=ttr_scr[:], in0=ebf_all[:, m, :], in1=wnb[:], op=ALU.mult
                )
                nc.vector.tensor_reduce(
                    out=svec[:, m:m + 1], in_=ttr_scr[:],
                    axis=mybir.AxisListType.X, op=ALU.add,
                )

            # prologue: first e unit + B(0) only.  Remaining e units are
            # pulled lazily inside block 0's m-loop so block-0 matmuls
            # interleave with the e stream (PE executes in program order).
            wtb0, wnba0 = b_alloc(0)
            multi = len(ctbs) > 1
            wtb1, wnba1 = b_alloc(1) if multi else (None, None)
            st0 = {}
            ncc0 = ctbs[0][1] // 128
            ncc1 = ctbs[1][1] // 128 if multi else 0
            # a small first unit so the first chunk's norm+transpose chain
            # starts ~3us earlier; the rest in 1MB units
            e_units = [(0, 2), (2, 4), (6, 4), (10, 4), (14, 2)]
            emb_state = {"eidx": 0, "ui": 0}

            def pull_emb(upto_m):
                while emb_state["eidx"] <= upto_m and emb_state["ui"] < len(e_units):
                    m0, n = e_units[emb_state["ui"]]
                    emb_unit(m0, n, emb_state["ui"])
                    emb_state["ui"] += 1
                    emb_state["eidx"] = m0 + n

            pull_emb(0)
            for cc in range(ncc0):
                b_load_norm(0, wnba0, cc, st0)
            for cc in range(ncc0):
                b_transpose(wnba0, wtb0, cc)

            # ---------- main loop: C(cb) with B(cb+1) interleaved ----------
            wsel_done = 0
            wsel_issued = 0

            def phi_block():
                s2 = smp.tile([128, mb], F32, tag="s2")
                nc.scalar.activation(
                    out=s2[:], in_=svec[:], func=AF.Square, bias=zb[:]
                )
                rl = smp.tile([128, mb], F32, tag="rl")
                nc.scalar.activation(
                    out=rl[:], in_=s2[:], func=AF.Relu, bias=s2b[:], scale=-1.0
                )
                sn = smp.tile([128, mb], F32, tag="sn")
                nc.scalar.activation(out=sn[:], in_=rl[:], func=AF.Sqrt, bias=zb[:])
                pc = smp.tile([128, mb], F32, tag="pc")
                nc.vector.tensor_scalar_mul(out=pc[:], in0=svec[:], scalar1=COS_M)
                smt = smp.tile([128, mb], F32, tag="smt")
                nc.vector.tensor_scalar_mul(out=smt[:], in0=sn[:], scalar1=SIN_M)
                ph = smp.tile([128, mb], F32, tag="ph")
                nc.vector.tensor_tensor(
                    out=ph[:], in0=pc[:], in1=smt[:], op=ALU.subtract
                )
                eb = smp.tile([128, mb], F32, tag="eb")
                nc.vector.tensor_scalar_add(
                    out=eb[:], in0=svec[:], scalar1=-SCALE * MM
                )
                mk = smp.tile([128, mb], mybir.dt.uint8, tag="mk")
                nc.vector.tensor_scalar(
                    out=mk[:], in0=svec[:], scalar1=SCALE * TH, scalar2=None,
                    op0=ALU.is_gt,
                )
                nc.vector.select(out=tval[:], mask=mk[:], on_true=ph[:], on_false=eb[:])
                nc.scalar.dma_start(out=tval_d[:], in_=tval[:])

            cur_wtb = wtb0
            cur_wnba = wnba0
            for cb_idx, (cb0, cbw) in enumerate(ctbs):
                last_cb = cb_idx == len(ctbs) - 1
                nxt_wtb = nxt_wnba = None
                nxt_state = {}
                nxt_ncc = 0
                if cb_idx == 0:
                    nxt_wtb, nxt_wnba = wtb1, wnba1
                    nxt_ncc = ncc1
                elif not last_cb:
                    nxt_wtb, nxt_wnba = b_alloc(cb_idx + 1)
                    nxt_ncc = ctbs[cb_idx + 1][1] // 128

                nps = (cbw + 1023) // 1024   # 1024-wide psum tiles (2 banks)
                ot = None
                for m in range(mb):
                    if cb_idx == 0:
                        # stay ~one DMA unit ahead of the matmuls on the e
                        # stream (ring credits throttle runaway prefetch)
                        pull_emb(m + 5)
                    if m % SGRP == 0:
                        ot = outp.tile(
                            [128, SGRP, max_cbw], BF16, tag="o_t",
                            name=f"ot_{cb_idx}_{m}",
                        )
                    n5 = (cbw + 511) // 512
                    pss = [
                        cpsum.tile([128, 512], F32, tag="mmps", name=f"mmps_{j}")
                        for j in range(n5)
                    ]
                    for k in range(kc):
                        for j5 in range(n5):
                            s0 = j5 * 512
                            sw = min(512, cbw - s0)
                            nc.tensor.matmul(
                                out=pss[j5][:, :sw],
                                lhsT=eT[:, k, m * 128:(m + 1) * 128],
                                rhs=cur_wtb[:, k, s0:s0 + sw],
                                start=(k == 0),
                                stop=(k == kc - 1),
                            )
                    for j in range(n5):
                        s0 = j * 512
                        sw = min(512, cbw - s0)
                        if evac_flip[0] % 2 == 0:
                            nc.vector.tensor_copy(
                                out=ot[:, m % SGRP, s0:s0 + sw], in_=pss[j][:, :sw]
                            )
                        else:
                            nc.scalar.copy(
                                out=ot[:, m % SGRP, s0:s0 + sw], in_=pss[j][:, :sw]
                            )
                        evac_flip[0] += 1
                    if m % SGRP == SGRP - 1:
                        # grouped store on the scalar HWDGE ring; the access
                        # pattern un-permutes rows (p,g) -> p*16+g
                        nc.scalar.dma_start(
                            out=out_pg[:, m - SGRP + 1:m + 1, cb0:cb0 + cbw],
                            in_=ot[:, :, :cbw],
                        )
                    # next block: loads+norms early (m 0..3).  PE transposes
                    # are emitted arrival-aware: the PE executes in order, so
                    # a transpose emitted before its chunk's DMA lands stalls
                    # the whole engine (and re-throttles it).  Start them only
                    # once the next block's bytes have had time to stream in.
                    if not last_cb:
                        if m < 4:
                            for cc in range(m * WGRP, min((m + 1) * WGRP, nxt_ncc)):
                                b_load_norm(cb_idx + 1, nxt_wnba, cc, nxt_state)
                        dma_ns = nxt_ncc * 873 + 5000
                        iter_ns = cbw * 1.71
                        m_start = max(4, int(dma_ns / iter_ns) + 1)
                        m_start = min(m_start, mb - (nxt_ncc + 1) // 2)
                        if m >= m_start:
                            # finish ~3 m-iterations before the block ends so
                            # the next block's first LDWEIGHTS never waits on
                            # the final transpose-copy chain
                            per_m = -(-nxt_ncc // max(1, mb - m_start - 3))
                            c0_ = (m - m_start) * per_m
                            for cc in range(c0_, min(c0_ + per_m, nxt_ncc)):
                                b_transpose(nxt_wnba, nxt_wtb, cc)
                    # sprinkle wsel chunks into the wide mid blocks (the ramp
                    # blocks' Act/DVE budget is fully claimed by norms+evacs);
                    # gathers run 2 chunks ahead of the norm+dot stage
                    if cb_idx >= 3 and m % 2 == 1:
                        if wsel_issued < mb:
                            wsel_gather(wsel_issued)
                            wsel_issued += 1
                        if wsel_done < wsel_issued - 2:
                            wsel_dot(wsel_done)
                            wsel_done += 1
                    if last_cb and m == 0:
                        while wsel_issued < mb:
                            wsel_gather(wsel_issued)
                            wsel_issued += 1
                        while wsel_done < mb:
                            wsel_dot(wsel_done)
                            wsel_done += 1
                        phi_block()
                cur_wtb = nxt_wtb
                cur_wnba = nxt_wnba

    nc.compile()
    return nc


_CACHE = {}


def _get_program():
    if "nc" not in _CACHE:
        _CACHE["nc"] = build_program()
    return _CACHE["nc"]


def make_in_maps(embeddings, labels, weight):
    embeddings = np.ascontiguousarray(np.asarray(embeddings, dtype=np.float32))
    weight = np.asarray(weight, dtype=np.float32)
    labels_np = np.asarray(labels).astype(np.int64)
    ident = np.eye(128, dtype=np.float32)
    # batch row r maps to (p, g) = (r // 16, r % 16)
    lab_pg = labels_np.reshape(128, B // 128)
    in_maps = []
    for k in range(N_CORES):
        wsh = np.zeros((CSP, D), np.float32)
        wsh[:CS] = weight[k * CS:(k + 1) * CS]
        own = (lab_pg // CS) == k
        col = lab_pg - k * CS
        goff_arr = np.ascontiguousarray(
            np.where(own, col, OOB).astype(np.int32)
        )
        in_maps.append(
            {"emb": embeddings, "wsh": wsh, "goff": goff_arr, "ident": ident}
        )
    return in_maps


def _gather(results, labels):
    labels_np = np.asarray(labels).astype(np.int64)
    full = np.empty((B, C), np.float32)
    for k in range(N_CORES):
        shard = np.asarray(results[k]["out"]).reshape(B, CSP)
        full[:, k * CS:(k + 1) * CS] = shard[:, :CS].astype(np.float32)
    # place the device-computed target values (64*phi) — indexing only
    rows = np.arange(B)
    cores = labels_np // CS
    for k in range(N_CORES):
        own = cores == k
        r = rows[own]
        tv = np.asarray(results[k]["tval"])  # [128, 16], row r at [r//16, r%16]
        full[r, labels_np[own]] = tv[r // 16, r % 16]
    return full


def kernel(embeddings, labels, weight):
    nc = _get_program()
    in_maps = make_in_maps(embeddings, labels, weight)
    res = run_bass_kernel_spmd(nc, in_maps, core_ids=list(range(N_CORES)))
    return _gather(res.results, labels)


def kernel_profiled(embeddings, labels, weight, **kw):
    """Like kernel() but also returns the BassKernelResults (exec_time_ns)."""
    nc = _get_program()
    in_maps = make_in_maps(embeddings, labels, weight)
    res = run_bass_kernel_spmd(
        nc, in_maps, core_ids=list(range(N_CORES)), trace=True, **kw
    )
    return _gather(res.results, labels), res
